# revision 16
# baseline (speedup 1.0000x reference)
import sys, os
for p in ("/opt/trn_rl_repo", "/root/.axon_site/_ro/trn_rl_repo"):
    if os.path.isdir(p) and p not in sys.path:
        sys.path.insert(0, p)

import numpy as np
import ml_dtypes

import concourse.bass as bass
import concourse.bacc as bacc
import concourse.tile as tile
from concourse import mybir
from concourse.bass_utils import run_bass_kernel_spmd

BF16 = ml_dtypes.bfloat16

# Problem constants (hardcoded per contract)
B, S, D = 2, 2048, 2048
HEADS, HD, NKV = 32, 64, 8
NCORES = 8
TPG = 4             # tensor-parallel groups per batch
HPC = HEADS // TPG  # 8 q-heads per core
KVPC = NKV // TPG   # 2 kv heads per core
NP = 4              # head pairs per core (kv0-head, kv1-head)
ST = S // 128       # 16 s-tiles
DT = D // 128       # 16 d_in-chunks
NPH = 4             # s-phases of 512
EPS = 1e-6

f32 = mybir.dt.float32
bf16 = mybir.dt.bfloat16

# offload PSUM-touching same-dtype elementwise work to gpsimd (Pool engine)
GPS_PSUM = False

_prog = None


def _build_program():
    nc = bacc.Bacc("TRN2", target_bir_lowering=False, debug=False)

    xT_d = nc.dram_tensor("xT", [D, S], bf16, kind="ExternalInput").ap()
    wqkv_d = nc.dram_tensor("wqkv", [D, 768], bf16, kind="ExternalInput").ap()
    wo_d = nc.dram_tensor("wo", [HPC * HD, D], bf16, kind="ExternalInput").ap()
    cosq_d = nc.dram_tensor("cosqT", [128, S], bf16, kind="ExternalInput").ap()
    sinq_d = nc.dram_tensor("sinqT", [128, S], bf16, kind="ExternalInput").ap()
    cosk_d = nc.dram_tensor("coskT", [128, S], bf16, kind="ExternalInput").ap()
    sink_d = nc.dram_tensor("sinkT", [128, S], bf16, kind="ExternalInput").ap()
    mdiag_d = nc.dram_tensor("mdiag", [128, 128], f32, kind="ExternalInput").ap()
    eind_d = nc.dram_tensor("eind", [128, 2], bf16, kind="ExternalInput").ap()
    eb_d = nc.dram_tensor("eb", [2, 128], bf16, kind="ExternalInput").ap()
    eb2_d = nc.dram_tensor("eb2", [64, 128], bf16, kind="ExternalInput").ap()
    psw_d = nc.dram_tensor("pswap", [128, 128], bf16, kind="ExternalInput").ap()
    id_d = nc.dram_tensor("id128", [128, 128], bf16, kind="ExternalInput").ap()
    out_d = nc.dram_tensor("out", [S, D], f32, kind="ExternalOutput").ap()

    with tile.TileContext(nc) as tc:
        with (
            tc.tile_pool(name="big", bufs=1) as big,
            tc.tile_pool(name="raw", bufs=5) as rawp,
            tc.tile_pool(name="sq", bufs=2) as sqp,
            tc.tile_pool(name="rv", bufs=2) as rvp,
            tc.tile_pool(name="t1", bufs=3) as t1p,
            tc.tile_pool(name="u", bufs=3) as up,
            tc.tile_pool(name="vt", bufs=2) as vtp,
            tc.tile_pool(name="pt", bufs=3) as ptp,
            tc.tile_pool(name="ys", bufs=3) as ysp,
            tc.tile_pool(name="slab", bufs=2, space="PSUM") as slabp,
            tc.tile_pool(name="ot", bufs=2, space="PSUM") as otp,
            tc.tile_pool(name="gp", bufs=2, space="PSUM") as gpp,
        ):
            # ---- resident SBUF tensors ----
            wqkv_sb = big.tile([128, DT, 768], bf16)
            nc.sync.dma_start(out=wqkv_sb, in_=wqkv_d.rearrange("(t p) n -> p t n", p=128))
            cosq_sb = big.tile([128, S], bf16)
            nc.sync.dma_start(out=cosq_sb, in_=cosq_d)
            sinq_sb = big.tile([128, S], bf16)
            nc.sync.dma_start(out=sinq_sb, in_=sinq_d)
            cosk_sb = big.tile([128, S], bf16)
            nc.sync.dma_start(out=cosk_sb, in_=cosk_d)
            sink_sb = big.tile([128, S], bf16)
            nc.sync.dma_start(out=sink_sb, in_=sink_d)
            mdiag_sb = big.tile([128, 128], f32)
            nc.sync.dma_start(out=mdiag_sb, in_=mdiag_d)
            eind_sb = big.tile([128, 2], bf16)
            nc.sync.dma_start(out=eind_sb, in_=eind_d)
            eb_sb = big.tile([2, 128], bf16)
            nc.sync.dma_start(out=eb_sb, in_=eb_d)
            eb2_sb = big.tile([64, 128], bf16)
            nc.sync.dma_start(out=eb2_sb, in_=eb2_d)
            psw_sb = big.tile([128, 128], bf16)
            nc.sync.dma_start(out=psw_sb, in_=psw_d)
            id_sb = big.tile([128, 128], bf16)
            nc.sync.dma_start(out=id_sb, in_=id_d)

            xT_sb = big.tile([128, DT, S], bf16)
            xT_r = xT_d.rearrange("(t p) s -> p t s", p=128)
            for d in range(DT):
                nc.sync.dma_start(out=xT_sb[:, d, :], in_=xT_r[:, d, :])
            wo_sb = big.tile([128, NP, D], bf16)
            nc.sync.dma_start(out=wo_sb, in_=wo_d.rearrange("(t p) n -> p t n", p=128))

            eps_sb = big.tile([2, 1], f32)
            nc.vector.memset(eps_sb, EPS)

            QT_sb = big.tile([128, NP, S], bf16)   # pair p: parts 0:64 head p, 64:128 head p+4
            KT_sb = big.tile([128, S], bf16)       # parts 0:64 kv0 dims, 64:128 kv1 dims
            Vbuf = big.tile([128, ST, 130], bf16)  # per s-tile: [64 v0 | 1 | 64 v1 | 1], ones at 64,129
            nc.vector.memset(Vbuf, 1.0)
            OT_sb = big.tile([128, NP, S], bf16)   # normalized O^T
            dn_sb = big.tile([64, 512], f32)       # denom staging: A at part 0, B at part 32
            nc.vector.memset(dn_sb, 1.0)
            rn_sb = big.tile([64, 512], bf16)

            ew = nc.gpsimd if GPS_PSUM else nc.vector

            for ph in range(NPH):
                sc = slice(ph * 512, (ph + 1) * 512)
                # ======== stage 1: projections for this 512-wide s window ========
                raws = []
                for c in range(6):
                    pj = gpp.tile([128, 512], f32, tag="gp")
                    for d in range(DT):
                        nc.tensor.matmul(pj[:], wqkv_sb[:, d, c * 128:(c + 1) * 128],
                                         xT_sb[:, d, sc], start=(d == 0), stop=(d == DT - 1))
                    if c == 5:
                        vt = vtp.tile([128, 512], bf16, tag="vt")
                        nc.vector.tensor_copy(vt[:], pj[:])
                        for st in range(4):
                            tp_ = gpp.tile([128, 128], bf16, tag="gp")
                            nc.tensor.transpose(tp_[:], vt[:, st * 128:(st + 1) * 128], id_sb[:])
                            nc.vector.tensor_copy(Vbuf[:, ph * 4 + st, 0:64], tp_[:, 0:64])
                            nc.vector.tensor_copy(Vbuf[:, ph * 4 + st, 65:129], tp_[:, 64:128])
                    else:
                        r = rawp.tile([128, 512], f32, tag="raw")
                        nc.vector.tensor_copy(r[:], pj[:])
                        raws.append(r)
                # ---- norm + rope for Q pairs (c=0..3) and K pair (c=4) ----
                for c in range(5):
                    r = raws[c]
                    sq = sqp.tile([128, 512], bf16, tag="sq")
                    nc.vector.tensor_mul(sq[:], r[:], r[:])
                    ssq = gpp.tile([2, 512], f32, tag="gp")
                    nc.tensor.matmul(ssq[:], eind_sb[:], sq[:], start=True, stop=True)
                    lnu = rvp.tile([2, 512], f32, tag="lnu")
                    nc.scalar.activation(lnu[:], ssq[:], mybir.ActivationFunctionType.Ln,
                                         bias=eps_sb[:], scale=1.0 / HD)
                    rv = rvp.tile([2, 512], bf16, tag="rv")
                    nc.scalar.activation(rv[:], lnu[:], mybir.ActivationFunctionType.Exp,
                                         scale=-0.5)
                    bc = gpp.tile([128, 512], f32, tag="gp")
                    nc.tensor.matmul(bc[:], eb_sb[:], rv[:], start=True, stop=True)
                    t1 = t1p.tile([128, 512], bf16, tag="t1")
                    nc.vector.tensor_mul(t1[:], r[:], bc[:])
                    cosT, sinT = (cosq_sb, sinq_sb) if c < 4 else (cosk_sb, sink_sb)
                    tm = up.tile([128, 512], bf16, tag="u")
                    nc.gpsimd.tensor_mul(tm[:], t1[:], cosT[:, sc])
                    u = up.tile([128, 512], bf16, tag="u")
                    nc.gpsimd.tensor_mul(u[:], t1[:], sinT[:, sc])
                    # partition-swap u (rotate-half) via PE permutation matmul
                    u2 = gpp.tile([128, 512], f32, tag="gp")
                    nc.tensor.matmul(u2[:], psw_sb[:], u[:], start=True, stop=True)
                    tgt = QT_sb[:, c, sc] if c < 4 else KT_sb[:, sc]
                    nc.vector.tensor_add(tgt, tm[:], u2[:])

                # ======== stage 2: attention for q-chunk ph ========
                cq = ph
                for p in range(NP):
                    otA = otp.tile([65, 512], f32, tag="ot")
                    otB = otp.tile([65, 512], f32, tag="ot")
                    for jg in range(2 * cq + 2):
                        slA = slabp.tile([128, 1024], f32, tag="slab")
                        slB = slabp.tile([128, 1024], f32, tag="slab")
                        for jj in range(2):
                            j = 2 * jg + jj
                            off = 128 * max(0, j - 4 * cq)
                            qs_ = QT_sb[:, p, cq * 512 + off:(cq + 1) * 512]
                            nc.tensor.matmul(slA[:, jj * 512 + off:(jj + 1) * 512],
                                             KT_sb[0:64, j * 128:(j + 1) * 128],
                                             qs_[0:64, :], start=True, stop=True)
                            nc.tensor.matmul(slB[:, jj * 512 + off:(jj + 1) * 512],
                                             KT_sb[64:128, j * 128:(j + 1) * 128],
                                             qs_[64:128, :], start=True, stop=True)
                            if j >= 4 * cq:
                                a0 = jj * 512 + off
                                ew.tensor_add(slA[:, a0:a0 + 128], slA[:, a0:a0 + 128],
                                              mdiag_sb[:])
                                ew.tensor_add(slB[:, a0:a0 + 128], slB[:, a0:a0 + 128],
                                              mdiag_sb[:])
                        pA = ptp.tile([128, 1024], bf16, tag="pt")
                        nc.scalar.activation(pA[:], slA[:], mybir.ActivationFunctionType.Exp,
                                             scale=0.125)
                        pB = ptp.tile([128, 1024], bf16, tag="pt")
                        nc.scalar.activation(pB[:], slB[:], mybir.ActivationFunctionType.Exp,
                                             scale=0.125)
                        for jj in range(2):
                            j = 2 * jg + jj
                            off = 128 * max(0, j - 4 * cq)
                            nc.tensor.matmul(otA[0:65, off:512], Vbuf[:, j, 0:65],
                                             pA[:, jj * 512 + off:(jj + 1) * 512],
                                             start=(j == 0), stop=(j == 4 * cq + 3))
                            nc.tensor.matmul(otB[0:65, off:512], Vbuf[:, j, 65:130],
                                             pB[:, jj * 512 + off:(jj + 1) * 512],
                                             start=(j == 0), stop=(j == 4 * cq + 3))
                    # ---- normalize: OT = ot[0:64] * (1/denom) ----
                    nc.vector.tensor_copy(dn_sb[0:1, :], otA[64:65, :])
                    nc.vector.tensor_copy(dn_sb[32:33, :], otB[64:65, :])
                    with nc.allow_low_precision("softmax denom reciprocal in bf16"):
                        nc.vector.reciprocal(rn_sb[:], dn_sb[:])
                    bc2 = gpp.tile([128, 512], f32, tag="gp")
                    nc.tensor.matmul(bc2[:], eb2_sb[:], rn_sb[:], start=True, stop=True)
                    stB = t1p.tile([128, 512], bf16, tag="stB", bufs=2)
                    nc.vector.tensor_copy(stB[0:64, :], otA[0:64, :])
                    nc.vector.tensor_copy(stB[64:128, :], otB[0:64, :])
                    nc.vector.tensor_mul(OT_sb[0:64, p, sc], stB[0:64, :], bc2[0:64, :])
                    nc.vector.tensor_mul(OT_sb[64:128, p, sc], stB[64:128, :], bc2[64:128, :])

                # ======== stage 3: output projection for q-tiles of this chunk ========
                for i in range(4 * cq, 4 * cq + 4):
                    for ns in range(4):
                        yp = gpp.tile([128, 512], f32, tag="gp")
                        for t in range(NP):
                            nc.tensor.matmul(yp[:], OT_sb[:, t, i * 128:(i + 1) * 128],
                                             wo_sb[:, t, ns * 512:(ns + 1) * 512],
                                             start=(t == 0), stop=(t == NP - 1))
                        ys = ysp.tile([128, 512], f32, tag="ys")
                        nc.vector.tensor_copy(ys[:], yp[:])
                        nc.sync.dma_start(out=out_d[i * 128:(i + 1) * 128,
                                                    ns * 512:(ns + 1) * 512], in_=ys[:])
    nc.compile()
    return nc


def _get_prog():
    global _prog
    if _prog is None:
        _prog = _build_program()
    return _prog


def _prep_inputs(x, mask, cos, sin, Wq, Wk, Wv, Wo, q_scale, k_scale):
    cos = np.asarray(cos, np.float32)
    sin = np.asarray(sin, np.float32)
    qs, ks = np.asarray(q_scale, np.float32), np.asarray(k_scale, np.float32)
    sgn = np.concatenate([-np.ones(32, np.float32), np.ones(32, np.float32)])

    def tables(w):
        wsw = np.concatenate([w[32:], w[:32]])
        cosT = (cos * w[None, :]).T                      # [64, S]
        sinf = sin * wsw[None, :] * sgn[None, :]         # [S, 64] folded
        sinT2 = np.concatenate([sinf[:, 32:], sinf[:, :32]], axis=1).T  # pre-swapped
        dup = lambda a: np.concatenate([a, a], axis=0).astype(BF16)     # [128, S]
        return dup(cosT), dup(sinT2)

    cosqT, sinqT = tables(qs)
    coskT, sinkT = tables(ks)

    k_ = np.arange(128)
    mdiag = np.where(k_[:, None] <= k_[None, :], 0.0, -1e9).astype(np.float32)
    eind = np.zeros((128, 2), np.float32)
    eind[0:64, 0] = 1.0
    eind[64:128, 1] = 1.0
    eb = np.zeros((2, 128), np.float32)
    eb[0, 0:64] = 1.0
    eb[1, 64:128] = 1.0
    eb2 = np.zeros((64, 128), np.float32)
    eb2[0, 0:64] = 1.0
    eb2[32, 64:128] = 1.0
    psw = np.zeros((128, 128), np.float32)
    for m in range(128):
        src = m + 32 if (m % 64) < 32 else m - 32
        psw[src, m] = 1.0

    in_maps = []
    for c in range(NCORES):
        b, g = c // TPG, c % TPG
        kvs = slice(g * KVPC * HD, (g + 1) * KVPC * HD)
        xT = np.ascontiguousarray(x[b].T).astype(BF16)
        cols = []
        for p in range(NP):
            cols.append(Wq[:, (g * HPC + p) * HD:(g * HPC + p + 1) * HD])
            cols.append(Wq[:, (g * HPC + p + 4) * HD:(g * HPC + p + 5) * HD])
        wqkv = np.concatenate(cols + [Wk[:, kvs], Wv[:, kvs]], axis=1).astype(BF16)
        ORD = [0, 4, 1, 5, 2, 6, 3, 7]
        wo = np.concatenate([Wo[(g * HPC + o) * HD:(g * HPC + o + 1) * HD, :] for o in ORD],
                            axis=0).astype(BF16)
        in_maps.append(dict(xT=xT, wqkv=wqkv, wo=wo,
                            cosqT=cosqT, sinqT=sinqT, coskT=coskT, sinkT=sinkT,
                            mdiag=mdiag, eind=eind.astype(BF16), eb=eb.astype(BF16),
                            eb2=eb2.astype(BF16), pswap=psw.astype(BF16),
                            id128=np.eye(128, dtype=np.float32).astype(BF16)))
    return in_maps


def kernel(x, mask, cos, sin, Wq, Wk, Wv, Wo, q_scale, k_scale, _trace=False):
    nc = _get_prog()
    in_maps = _prep_inputs(x, mask, cos, sin, Wq, Wk, Wv, Wo, q_scale, k_scale)
    res = run_bass_kernel_spmd(nc, in_maps, core_ids=list(range(NCORES)), trace=_trace)
    kernel.last_results = res
    out = np.zeros((B, S, D), np.float32)
    for c in range(NCORES):
        out[c // TPG] += res.results[c]["out"]
    return out


# revision 22
# speedup vs baseline: 1.1206x; 1.1206x over previous
import sys, os
for p in ("/opt/trn_rl_repo", "/root/.axon_site/_ro/trn_rl_repo"):
    if os.path.isdir(p) and p not in sys.path:
        sys.path.insert(0, p)

import numpy as np
import ml_dtypes

import concourse.bass as bass
import concourse.bacc as bacc
import concourse.tile as tile
from concourse import mybir
from concourse.bass_utils import run_bass_kernel_spmd

BF16 = ml_dtypes.bfloat16

# Problem constants (hardcoded per contract)
B, S, D = 2, 2048, 2048
HEADS, HD, NKV = 32, 64, 8
NCORES = 8
TPG = 4             # tensor-parallel groups per batch
HPC = HEADS // TPG  # 8 q-heads per core
KVPC = NKV // TPG   # 2 kv heads per core
NP = 4              # head pairs per core (kv0-head, kv1-head)
ST = S // 128       # 16 s-tiles
DT = D // 128       # 16 d_in-chunks
NPH = 4             # s-phases of 512
EPS = 1e-6

f32 = mybir.dt.float32
bf16 = mybir.dt.bfloat16

# offload PSUM-touching same-dtype elementwise work to gpsimd (Pool engine)
GPS_PSUM = False

_prog = None


def _build_program():
    nc = bacc.Bacc("TRN2", target_bir_lowering=False, debug=False)

    xT_d = nc.dram_tensor("xT", [D, S], bf16, kind="ExternalInput").ap()
    wqkv_d = nc.dram_tensor("wqkv", [D, 768], bf16, kind="ExternalInput").ap()
    wo_d = nc.dram_tensor("wo", [HPC * HD, D], bf16, kind="ExternalInput").ap()
    cos_d = nc.dram_tensor("cosT", [128, S], bf16, kind="ExternalInput").ap()
    sin_d = nc.dram_tensor("sinT2", [128, S], bf16, kind="ExternalInput").ap()
    mdiag_d = nc.dram_tensor("mdiagT", [128, 128], bf16, kind="ExternalInput").ap()
    eind_d = nc.dram_tensor("eind", [128, 2], bf16, kind="ExternalInput").ap()
    ebq_d = nc.dram_tensor("ebq", [2, 128], bf16, kind="ExternalInput").ap()
    ebk_d = nc.dram_tensor("ebk", [2, 128], bf16, kind="ExternalInput").ap()
    eb2_d = nc.dram_tensor("eb2", [64, 128], bf16, kind="ExternalInput").ap()
    psw_d = nc.dram_tensor("pswap", [128, 128], bf16, kind="ExternalInput").ap()
    id_d = nc.dram_tensor("id128", [128, 128], bf16, kind="ExternalInput").ap()
    out_d = nc.dram_tensor("out", [S, D], f32, kind="ExternalOutput").ap()

    with tile.TileContext(nc) as tc:
        with (
            tc.tile_pool(name="big", bufs=1) as big,
            tc.tile_pool(name="raw", bufs=5) as rawp,
            tc.tile_pool(name="sq", bufs=2) as sqp,
            tc.tile_pool(name="rv", bufs=2) as rvp,
            tc.tile_pool(name="t1", bufs=2) as t1p,
            tc.tile_pool(name="u", bufs=2) as up,
            tc.tile_pool(name="vt", bufs=2) as vtp,
            tc.tile_pool(name="pt", bufs=4) as ptp,
            tc.tile_pool(name="ys", bufs=2) as ysp,
            tc.tile_pool(name="slab", bufs=2, space="PSUM") as slabp,
            tc.tile_pool(name="ot", bufs=2, space="PSUM") as otp,
            tc.tile_pool(name="gp", bufs=2, space="PSUM") as gpp,
        ):
            # ---- resident SBUF tensors ----
            wqkv_sb = big.tile([128, DT, 768], bf16)
            nc.sync.dma_start(out=wqkv_sb, in_=wqkv_d.rearrange("(t p) n -> p t n", p=128))
            cos_sb = big.tile([128, S], bf16)
            nc.sync.dma_start(out=cos_sb, in_=cos_d)
            sin_sb = big.tile([128, S], bf16)
            nc.sync.dma_start(out=sin_sb, in_=sin_d)
            mdiagT_sb = big.tile([128, 128], bf16)
            nc.sync.dma_start(out=mdiagT_sb, in_=mdiag_d)
            eind_sb = big.tile([128, 2], bf16)
            nc.sync.dma_start(out=eind_sb, in_=eind_d)
            ebq_sb = big.tile([2, 128], bf16)
            nc.sync.dma_start(out=ebq_sb, in_=ebq_d)
            ebk_sb = big.tile([2, 128], bf16)
            nc.sync.dma_start(out=ebk_sb, in_=ebk_d)
            eb2_sb = big.tile([64, 128], bf16)
            nc.sync.dma_start(out=eb2_sb, in_=eb2_d)
            psw_sb = big.tile([128, 128], bf16)
            nc.sync.dma_start(out=psw_sb, in_=psw_d)
            id_sb = big.tile([128, 128], bf16)
            nc.sync.dma_start(out=id_sb, in_=id_d)

            xT_sb = big.tile([128, DT, S], bf16)
            xT_r = xT_d.rearrange("(t p) s -> p t s", p=128)
            for d in range(DT):
                nc.sync.dma_start(out=xT_sb[:, d, :], in_=xT_r[:, d, :])
            wo_sb = big.tile([128, NP, D], bf16)
            nc.sync.dma_start(out=wo_sb, in_=wo_d.rearrange("(t p) n -> p t n", p=128))

            eps_sb = big.tile([2, 1], f32)
            nc.vector.memset(eps_sb, EPS)

            QT_sb = big.tile([128, NP, S], bf16)   # pair p: parts 0:64 head p, 64:128 head p+4
            KT_sb = big.tile([128, S], bf16)       # parts 0:64 kv0 dims, 64:128 kv1 dims
            Vbuf = big.tile([128, ST, 130], bf16)  # per s-tile: [64 v0 | 1 | 64 v1 | 1], ones at 64,129
            nc.vector.memset(Vbuf, 1.0)
            OT_sb = big.tile([128, NP, S], bf16)   # normalized O^T
            dn_sb = big.tile([64, 512], f32)       # denom staging: A at part 0, B at part 32
            nc.vector.memset(dn_sb, 1.0)
            rnf_sb = big.tile([64, 512], f32)
            rn_sb = big.tile([64, 512], bf16)

            ew = nc.gpsimd if GPS_PSUM else nc.vector

            def stage3(cq):
                # output projection for q-tiles of chunk cq
                for i in range(4 * cq, 4 * cq + 4):
                    for ns in range(4):
                        yp = gpp.tile([128, 512], f32, tag="gp")
                        for t in range(NP):
                            nc.tensor.matmul(yp[:], OT_sb[:, t, i * 128:(i + 1) * 128],
                                             wo_sb[:, t, ns * 512:(ns + 1) * 512],
                                             start=(t == 0), stop=(t == NP - 1))
                        ys = ysp.tile([128, 512], f32, tag="ys")
                        nc.vector.tensor_copy(ys[:], yp[:])
                        nc.sync.dma_start(out=out_d[i * 128:(i + 1) * 128,
                                                    ns * 512:(ns + 1) * 512], in_=ys[:])

            for ph in range(NPH):
                sc = slice(ph * 512, (ph + 1) * 512)
                # ======== stage 1: projections for this 512-wide s window ========
                raws = []
                for c in range(6):
                    pj = gpp.tile([128, 512], f32, tag="gp")
                    for d in range(DT):
                        nc.tensor.matmul(pj[:], wqkv_sb[:, d, c * 128:(c + 1) * 128],
                                         xT_sb[:, d, sc], start=(d == 0), stop=(d == DT - 1))
                    if c == 5:
                        vt = vtp.tile([128, 512], bf16, tag="vt")
                        nc.vector.tensor_copy(vt[:], pj[:])
                        for st in range(4):
                            tp_ = gpp.tile([128, 128], bf16, tag="gp")
                            nc.tensor.transpose(tp_[:], vt[:, st * 128:(st + 1) * 128], id_sb[:])
                            nc.vector.tensor_copy(Vbuf[:, ph * 4 + st, 0:64], tp_[:, 0:64])
                            nc.vector.tensor_copy(Vbuf[:, ph * 4 + st, 65:129], tp_[:, 64:128])
                    else:
                        r = rawp.tile([128, 512], f32, tag="raw")
                        nc.vector.tensor_copy(r[:], pj[:])
                        raws.append(r)
                # ---- norm + rope: batched so ACT runs Ln x5 then one Exp (2 table loads) ----
                lnu = rvp.tile([2, 2560], f32, tag="lnu", bufs=1)
                for c in range(5):
                    sq = sqp.tile([128, 512], bf16, tag="sq")
                    nc.vector.tensor_mul(sq[:], raws[c][:], raws[c][:])
                    ssq = gpp.tile([2, 512], f32, tag="gp")
                    nc.tensor.matmul(ssq[:], eind_sb[:], sq[:], start=True, stop=True)
                    nc.scalar.activation(lnu[:, c * 512:(c + 1) * 512], ssq[:],
                                         mybir.ActivationFunctionType.Ln,
                                         bias=eps_sb[:], scale=1.0 / HD)
                rv = rvp.tile([2, 2560], bf16, tag="rv", bufs=1)
                nc.scalar.activation(rv[:], lnu[:], mybir.ActivationFunctionType.Exp,
                                     scale=-0.5)
                for c in range(5):
                    bc = gpp.tile([128, 512], f32, tag="gp")
                    nc.tensor.matmul(bc[:], ebq_sb[:] if c < 4 else ebk_sb[:],
                                     rv[:, c * 512:(c + 1) * 512], start=True, stop=True)
                    t1 = t1p.tile([128, 512], bf16, tag="t1")
                    nc.vector.tensor_mul(t1[:], raws[c][:], bc[:])
                    tm = up.tile([128, 512], bf16, tag="u")
                    nc.gpsimd.tensor_mul(tm[:], t1[:], cos_sb[:, sc])
                    u = up.tile([128, 512], bf16, tag="u")
                    nc.gpsimd.tensor_mul(u[:], t1[:], sin_sb[:, sc])
                    # partition-swap u (rotate-half) via PE permutation matmul
                    u2 = gpp.tile([128, 512], f32, tag="gp")
                    nc.tensor.matmul(u2[:], psw_sb[:], u[:], start=True, stop=True)
                    tgt = QT_sb[:, c, sc] if c < 4 else KT_sb[:, sc]
                    nc.vector.tensor_add(tgt, tm[:], u2[:])

                # ---- stage 3 of previous chunk fills PE while ACT finishes norm ----
                if ph >= 1:
                    stage3(ph - 1)

                # ======== stage 2: attention for q-chunk ph ========
                cq = ph
                for p in range(NP):
                    otA = otp.tile([65, 512], f32, tag="ot")
                    otB = otp.tile([65, 512], f32, tag="ot")
                    prev = None
                    for jg in range(2 * cq + 2):
                        slA = slabp.tile([128, 1024], f32, tag="slab")
                        slB = slabp.tile([128, 1024], f32, tag="slab")
                        for jj in range(2):
                            j = 2 * jg + jj
                            off = 128 * max(0, j - 4 * cq)
                            diag = j >= 4 * cq
                            qs_ = QT_sb[:, p, cq * 512 + off:(cq + 1) * 512]
                            nc.tensor.matmul(slA[:, jj * 512 + off:(jj + 1) * 512],
                                             KT_sb[0:64, j * 128:(j + 1) * 128],
                                             qs_[0:64, :], start=True, stop=not diag,
                                             skip_group_check=diag)
                            nc.tensor.matmul(slB[:, jj * 512 + off:(jj + 1) * 512],
                                             KT_sb[64:128, j * 128:(j + 1) * 128],
                                             qs_[64:128, :], start=True, stop=not diag,
                                             skip_group_check=diag)
                            if diag:
                                a0 = jj * 512 + off
                                nc.tensor.matmul(slA[:, a0:a0 + 128], mdiagT_sb[:], id_sb[:],
                                                 start=False, stop=True, skip_group_check=True)
                                nc.tensor.matmul(slB[:, a0:a0 + 128], mdiagT_sb[:], id_sb[:],
                                                 start=False, stop=True, skip_group_check=True)
                        pA = ptp.tile([128, 1024], bf16, tag="pt")
                        nc.scalar.activation(pA[:], slA[:], mybir.ActivationFunctionType.Exp,
                                             scale=0.125)
                        pB = ptp.tile([128, 1024], bf16, tag="pt")
                        nc.scalar.activation(pB[:], slB[:], mybir.ActivationFunctionType.Exp,
                                             scale=0.125)
                        if prev is not None:
                            for jj in range(2):
                                j = 2 * prev[0] + jj
                                off = 128 * max(0, j - 4 * cq)
                                nc.tensor.matmul(otA[0:65, off:512], Vbuf[:, j, 0:65],
                                                 prev[1][:, jj * 512 + off:(jj + 1) * 512],
                                                 start=(j == 0), stop=(j == 4 * cq + 3))
                                nc.tensor.matmul(otB[0:65, off:512], Vbuf[:, j, 65:130],
                                                 prev[2][:, jj * 512 + off:(jj + 1) * 512],
                                                 start=(j == 0), stop=(j == 4 * cq + 3))
                        prev = (jg, pA, pB)
                    for jj in range(2):
                        j = 2 * prev[0] + jj
                        off = 128 * max(0, j - 4 * cq)
                        nc.tensor.matmul(otA[0:65, off:512], Vbuf[:, j, 0:65],
                                         prev[1][:, jj * 512 + off:(jj + 1) * 512],
                                         start=(j == 0), stop=(j == 4 * cq + 3))
                        nc.tensor.matmul(otB[0:65, off:512], Vbuf[:, j, 65:130],
                                         prev[2][:, jj * 512 + off:(jj + 1) * 512],
                                         start=(j == 0), stop=(j == 4 * cq + 3))
                    # ---- normalize: OT = ot[0:64] * (1/denom) ----
                    nc.vector.tensor_copy(dn_sb[0:1, :], otA[64:65, :])
                    nc.vector.tensor_copy(dn_sb[32:33, :], otB[64:65, :])
                    nc.vector.reciprocal_approx_fast(rnf_sb[:], dn_sb[:])
                    with nc.allow_low_precision("bf16 cast of softmax denom recip"):
                        nc.vector.tensor_copy(rn_sb[:], rnf_sb[:])
                    bc2 = gpp.tile([128, 512], f32, tag="gp")
                    nc.tensor.matmul(bc2[:], eb2_sb[:], rn_sb[:], start=True, stop=True)
                    stB = t1p.tile([128, 512], bf16, tag="stB", bufs=2)
                    nc.vector.tensor_copy(stB[0:64, :], otA[0:64, :])
                    nc.vector.tensor_copy(stB[64:128, :], otB[0:64, :])
                    nc.vector.tensor_mul(OT_sb[0:64, p, sc], stB[0:64, :], bc2[0:64, :])
                    nc.vector.tensor_mul(OT_sb[64:128, p, sc], stB[64:128, :], bc2[64:128, :])

            stage3(NPH - 1)
    nc.compile()
    return nc


def _get_prog():
    global _prog
    if _prog is None:
        _prog = _build_program()
    return _prog


def _prep_inputs(x, mask, cos, sin, Wq, Wk, Wv, Wo, q_scale, k_scale):
    cos = np.asarray(cos, np.float32)
    sin = np.asarray(sin, np.float32)
    qs, ks = np.asarray(q_scale, np.float32), np.asarray(k_scale, np.float32)
    sgn = np.concatenate([-np.ones(32, np.float32), np.ones(32, np.float32)])

    dup = lambda a: np.concatenate([a, a], axis=0).astype(BF16)      # [128, S]
    cosT = dup(cos.T)
    # sinT2[e] = sin[dst(e)] * sgn[dst(e)], dst(e) = partner dim of e
    sinT2 = dup(np.concatenate([sin[:, 32:], -sin[:, :32]], axis=1).T)

    k_ = np.arange(128)
    mdiagT = np.where(k_[:, None] < k_[None, :], -1e9, 0.0).astype(np.float32)
    eind = np.zeros((128, 2), np.float32)
    eind[0:64, 0] = 1.0
    eind[64:128, 1] = 1.0
    ebq = np.zeros((2, 128), np.float32)
    ebq[0, 0:64] = qs
    ebq[1, 64:128] = qs
    ebk = np.zeros((2, 128), np.float32)
    ebk[0, 0:64] = ks
    ebk[1, 64:128] = ks
    eb2 = np.zeros((64, 128), np.float32)
    eb2[0, 0:64] = 1.0
    eb2[32, 64:128] = 1.0
    psw = np.zeros((128, 128), np.float32)
    for m in range(128):
        src = m + 32 if (m % 64) < 32 else m - 32
        psw[src, m] = 1.0

    in_maps = []
    for c in range(NCORES):
        b, g = c // TPG, c % TPG
        kvs = slice(g * KVPC * HD, (g + 1) * KVPC * HD)
        xT = np.ascontiguousarray(x[b].T).astype(BF16)
        cols = []
        for p in range(NP):
            cols.append(Wq[:, (g * HPC + p) * HD:(g * HPC + p + 1) * HD])
            cols.append(Wq[:, (g * HPC + p + 4) * HD:(g * HPC + p + 5) * HD])
        wqkv = np.concatenate(cols + [Wk[:, kvs], Wv[:, kvs]], axis=1).astype(BF16)
        ORD = [0, 4, 1, 5, 2, 6, 3, 7]
        wo = np.concatenate([Wo[(g * HPC + o) * HD:(g * HPC + o + 1) * HD, :] for o in ORD],
                            axis=0).astype(BF16)
        in_maps.append(dict(xT=xT, wqkv=wqkv, wo=wo, cosT=cosT, sinT2=sinT2,
                            mdiagT=mdiagT.astype(BF16), eind=eind.astype(BF16),
                            ebq=ebq.astype(BF16), ebk=ebk.astype(BF16),
                            eb2=eb2.astype(BF16), pswap=psw.astype(BF16),
                            id128=np.eye(128, dtype=np.float32).astype(BF16)))
    return in_maps


def kernel(x, mask, cos, sin, Wq, Wk, Wv, Wo, q_scale, k_scale, _trace=False):
    nc = _get_prog()
    in_maps = _prep_inputs(x, mask, cos, sin, Wq, Wk, Wv, Wo, q_scale, k_scale)
    res = run_bass_kernel_spmd(nc, in_maps, core_ids=list(range(NCORES)), trace=_trace)
    kernel.last_results = res
    out = np.zeros((B, S, D), np.float32)
    for c in range(NCORES):
        out[c // TPG] += res.results[c]["out"]
    return out
